# revision 52
# baseline (speedup 1.0000x reference)
"""Trainium2 Bass kernel for an MoE classification head.

Model (per reference):
    normed = LayerNorm(x)  (no affine; shared across gate+experts)
    gate   = softmax((normed * g_g + b_g) @ gate_w + gate_b)     [B, E]
    h_e    = GELU((normed * g_e + b_e) @ w1_e + b1_e)            [E, B, H]
    out    = sum_e gate[:, e] * (h_e @ w2_e + b2_e)              [B, C]

Strategy: data-parallel over 8 NeuronCores (batch sharded 2048 rows/core,
all parameters replicated).  Per-expert LayerNorm affines fold into w1/b1
on the host; everything runs in bf16 on the PE (error budget 2e-2 >> bf16
noise ~5e-3).  Key device-side structure:

  phase 0  per 128-row tile: DMA x (bf16) -> bn_stats/bn_aggr for LN
           stats -> scalar normalize -> XBAR DMA-transpose into
           normedT [128, KC, BS] (no PE transposes, no PSUM copies).
  gate     transposed logits [E, 512] via PE (stationary gw, moving
           normedT), exp on scalar, Sum/replication via tiny PE matmuls
           (ones / replication matrices); softmax denominator applied at
           the very end (out *= 1/sum replicated to C partitions).
  experts  for e, hc: mm1 accumulates KC chunks into 4 psum banks
           (batch chunks of 512); GELU+b1 on scalar -> bf16 hT; mm2
           accumulates over hc into [C, 512] psum.  Gated sum via one
           fused scalar_tensor_tensor: (mm2 + b2) * gate_exp.

A post-scheduling pass dedups consecutive identical LDWEIGHTS (the 4
batch-chunk matmuls per weight tile share one weight load).
"""

import os
import numpy as np
import ml_dtypes

import concourse.bacc as bacc
import concourse.mybir as mybir
from concourse import tile
from concourse.bass_utils import run_bass_kernel_spmd

F32 = mybir.dt.float32
F32R = mybir.dt.float32r
BF16 = mybir.dt.bfloat16
AF = mybir.ActivationFunctionType
ALU = mybir.AluOpType

N_CORES = 8
B, D, H, E, C = 16384, 1024, 2048, 4, 3
BS = B // N_CORES       # 2048 rows per core
NT = BS // 128          # 16 batch tiles of 128 rows
KC = D // 128           # 8 contraction chunks over D
NBC = BS // 512         # 4 batch chunks of 512 (matmul moving dim)
NHC = H // 128          # 16 H chunks
EPS = 1e-5
BFNP = ml_dtypes.bfloat16

_NC_CACHE = {}
# HW-unsafe: with one LDWEIGHTS per 4 matmuls the PE reorder window pulls
# the next weight load ahead and clobbers the array for the later matmuls
# of the group (bc2/bc3 outputs corrupt).  Measured benefit was ~0.3%, the
# reorder window already hides duplicate loads.  Keep off.
LDW_DEDUP = os.environ.get("BASS_LDW_DEDUP", "") != ""


def _dedup_ldweights(nc):
    """Remove back-to-back InstLdweights that reload identical weights.

    The tile scheduler splits every matmul into LDWEIGHTS + MATMUL; when
    consecutive matmuls share the same stationary operand (our 4 batch
    chunks per weight tile) the repeat loads are pure overhead on the PE.
    Only sync-free duplicates are dropped, so all semaphore waits are
    preserved.
    """
    removed = 0
    for f in nc.m.functions:
        for blk in f.blocks:
            insts = list(blk.instructions)
            keep = []
            last_sig = None
            pending_waits = []
            changed = False
            for i in insts:
                if isinstance(i, mybir.InstLdweights):
                    si = i.sync_info
                    clean = si is None or (
                        len(si.on_wait) == 0 and len(si.on_update) == 0
                    )
                    sig = (
                        str(i.ins),
                        str(i.perf_mode),
                        str(i.is_transpose),
                        str(i.tile_position),
                    )
                    if clean and sig == last_sig:
                        removed += 1
                        changed = True
                        continue
                    last_sig = sig
                keep.append(i)
            if changed:
                blk.instructions = keep
    return removed


def _build_nc():
    nc = bacc.Bacc("TRN2", target_bir_lowering=False, debug=False,
                   enable_asserts=False, num_devices=N_CORES)
    x = nc.dram_tensor("x", [BS, D], BF16, kind="ExternalInput")
    gw = nc.dram_tensor("gw", [128, KC, E], BF16, kind="ExternalInput")
    gb = nc.dram_tensor("gb", [E, 1], F32, kind="ExternalInput")
    w1 = nc.dram_tensor("w1", [E, 128, KC, H], BF16, kind="ExternalInput")
    b1 = nc.dram_tensor("b1", [E, 128, NHC], F32, kind="ExternalInput")
    w2 = nc.dram_tensor("w2", [E, 128, NHC * C], BF16, kind="ExternalInput")
    b2 = nc.dram_tensor("b2", [128, E], F32, kind="ExternalInput")
    rep = nc.dram_tensor("rep", [E, E * NBC * 128], F32, kind="ExternalInput")
    on4 = nc.dram_tensor("on4", [E, 1], F32, kind="ExternalInput")
    on13 = nc.dram_tensor("on13", [1, NBC * 128], F32, kind="ExternalInput")
    y = nc.dram_tensor("y", [C, BS], F32, kind="ExternalOutput")

    with tile.TileContext(nc) as tc:
        with (
            tc.tile_pool(name="pers", bufs=1) as pers,
            tc.tile_pool(name="xp", bufs=3) as xp,
            tc.tile_pool(name="nrmp", bufs=2) as nrmp,
            tc.tile_pool(name="st", bufs=3) as st,
            tc.tile_pool(name="gp", bufs=2) as gp,
            tc.tile_pool(name="w1p", bufs=2) as w1p,
            tc.tile_pool(name="ep", bufs=2) as ep,
            tc.tile_pool(name="hp", bufs=4) as hp,
            tc.tile_pool(name="php", bufs=4, space="PSUM") as php,
            tc.tile_pool(name="plp", bufs=4, space="PSUM") as plp,
        ):
            # ---- persistent tiles ----
            normedT = pers.tile([128, KC, BS], BF16)  # normalized x, transposed
            gwsb = pers.tile([128, KC, E], BF16)
            gbsb = pers.tile([E, 1], F32)
            b2sb = pers.tile([128, E], F32)           # b2 striped at 32*bc+c
            accT = pers.tile([128, BS], F32)          # gated sum, striped
            gexps = [pers.tile([128, BS], BF16, name=f"gexp{e}")
                     for e in range(E)]               # exp(logit_e), striped
            rin = pers.tile([1, BS], F32R)            # 1/sum_e exp
            rrep = pers.tile([128, BS], F32)          # rin striped to 32*bc+c
            epst = pers.tile([128, 1], F32)
            ones4 = pers.tile([E, 1], F32R)           # column of ones (sum_e)
            ones13 = pers.tile([1, NBC * 128], F32R)  # striped 1s per bc
            repm = pers.tile([E, E * NBC * 128], F32R)  # striped replication

            nc.vector.memset(epst[:], EPS)
            nc.gpsimd.memset(accT[:], 0.0)
            nc.gpsimd.memset(rrep[:], 0.0)
            nc.scalar.dma_start(ones4[:], on4[:].bitcast(F32R))
            nc.scalar.dma_start(ones13[:], on13[:].bitcast(F32R))
            nc.scalar.dma_start(repm[:], rep[:].bitcast(F32R))
            nc.scalar.dma_start(gwsb[:], gw[:])
            nc.scalar.dma_start(gbsb[:], gb[:])
            nc.scalar.dma_start(b2sb[:], b2[:])

            # prefetch expert 0 weights (gpsimd queue is otherwise idle)
            w1sb0 = w1p.tile([128, KC, H], BF16, tag="w1", name="w1sb0")
            nc.gpsimd.dma_start(w1sb0[:], w1[0])
            b1sb0 = ep.tile([128, NHC], F32, tag="b1", name="b1sb0")
            nc.gpsimd.dma_start(b1sb0[:], b1[0])
            w2sb0 = ep.tile([128, NHC * C], BF16, tag="w2", name="w2sb0")
            nc.gpsimd.dma_start(w2sb0[:], w2[0])

            # ---- phase 0: LayerNorm stats + normalize + DMA-transpose ----
            # two row-tiles per load so the tiny stats ops batch 2 wide
            for tp in range(NT // 2):
                r0 = 256 * tp
                xt = xp.tile([128, 2, D], BF16, tag="xt", name="xt", bufs=8)
                nc.sync.dma_start(
                    xt[:], x[r0:r0 + 256, :].rearrange("(a p) d -> p a d",
                                                       p=128))
                mvp = st.tile([128, 2, 2], F32, tag="mvp")
                for i in range(2):
                    bnst = st.tile([128, 12], F32, tag="bnst")
                    nc.vector.bn_stats(bnst[:, 0:6], xt[:, i, 0:512])
                    nc.vector.bn_stats(bnst[:, 6:12], xt[:, i, 512:1024])
                    nc.vector.bn_aggr(mvp[:, i, :], bnst[:])
                sdp = st.tile([128, 2], F32, tag="sdp")
                nc.scalar.activation(sdp[:], mvp[:, :, 1], AF.Sqrt,
                                     bias=epst[:])
                rsp = st.tile([128, 2], F32, tag="rsp")
                nc.vector.reciprocal(rsp[:], sdp[:])
                nmp = st.tile([128, 2], F32, tag="nmp")
                nc.vector.scalar_tensor_tensor(
                    nmp[:], mvp[:, :, 0], -1.0, rsp[:], ALU.mult, ALU.mult)
                for i in range(2):
                    ti = 2 * tp + i
                    bsl = slice(ti * 128, (ti + 1) * 128)
                    # normalize in place (elementwise, same offsets): the
                    # nrm intermediate is gone, and with 8 xt buffers (no
                    # reuse at all) the x DMA issues never block the sync
                    # queue that also carries the transposes
                    nc.scalar.activation(xt[:, i, :], xt[:, i, :],
                                         AF.Identity, bias=nmp[:, i:i + 1],
                                         scale=rsp[:, i:i + 1])
                    # [128b, 1024d] -> normedT[dp, kc, b] via XBAR transpose
                    nc.sync.dma_start(normedT[:, :, bsl], xt[:, i, :],
                                      transpose=True)

            # ---- gate: logitsT -> exp -> sum + replicate (PE) ----
            # batch the PE work so the in-order PE queue never stalls on the
            # scalar/vector softmax chain.  gb2 = gbsb + 0*normedT[last tile]
            # pins the exps after ALL phase-0 scalar work, so the act table
            # switches sqrt-set -> exp-set -> gelu-set exactly once each.
            gb2 = pers.tile([E, 1], F32)
            nc.vector.scalar_tensor_tensor(
                gb2[:], normedT[0:E, KC - 1, BS - 1:BS], 0.0, gbsb[:],
                ALU.mult, ALU.add)
            pgs, exgs = [], []
            for bc in range(NBC):
                csl = slice(bc * 512, (bc + 1) * 512)
                pg = php.tile([E, 512], F32, tag="mm", name="pg")
                for kc in range(KC):
                    nc.tensor.matmul(pg[:], gwsb[:, kc, :], normedT[:, kc, csl],
                                     start=(kc == 0), stop=(kc == KC - 1))
                pgs.append(pg)
            for bc in range(NBC):
                exg = gp.tile([E, 512], F32R, tag="exg", name="exg", bufs=4)
                nc.scalar.activation(exg[:], pgs[bc][:], AF.Exp, bias=gb2[:])
                exgs.append(exg)
            for bc in range(NBC):
                csl = slice(bc * 512, (bc + 1) * 512)
                ps1 = plp.tile([1, 512], F32, tag="pl", name="ps1")
                nc.tensor.matmul(ps1[:], ones4[:], exgs[bc][:],
                                 start=True, stop=True)
                with nc.allow_low_precision(reason="f32r output is f32 bits"):
                    nc.vector.reciprocal(rin[:, csl], ps1[:])
                for e in range(E):
                    off = (e * NBC + bc) * 128
                    pool = php if e % 2 == 0 else plp
                    tag = "mm" if e % 2 == 0 else "pl"
                    pr = pool.tile([99, 512], F32, tag=tag, name="pr")
                    nc.tensor.matmul(
                        pr[:], repm[:, off:off + 99],
                        exgs[bc][:], start=True, stop=True)
                    nc.vector.tensor_copy(gexps[e][0:99, csl], pr[:])

            # ---- experts ----
            for e in range(E):
                if e == 0:
                    w1sb, b1sb, w2sb = w1sb0, b1sb0, w2sb0
                else:
                    w1sb = w1p.tile([128, KC, H], BF16, tag="w1")
                    nc.gpsimd.dma_start(w1sb[:], w1[e])
                    b1sb = ep.tile([128, NHC], F32, tag="b1")
                    nc.gpsimd.dma_start(b1sb[:], b1[e])
                    w2sb = ep.tile([128, NHC * C], BF16, tag="w2")
                    nc.gpsimd.dma_start(w2sb[:], w2[e])

                pls = [plp.tile([99, 512], F32, tag="pl", name="pl")
                       for _ in range(NBC)]
                # mm2 for iteration hc is issued during mm1 of hc+1, so it
                # never waits on a just-issued GELU at the PE queue head
                prev_hT = None
                for hc in range(NHC):
                    hsl = slice(hc * 128, (hc + 1) * 128)
                    phs = [php.tile([128, 512], F32, tag="mm", name="ph")
                           for _ in range(NBC)]
                    for kc in range(KC):
                        for bc in range(NBC):
                            nc.tensor.matmul(
                                phs[bc][:], w1sb[:, kc, hsl],
                                normedT[:, kc, bc * 512:(bc + 1) * 512],
                                start=(kc == 0), stop=(kc == KC - 1))
                    if prev_hT is not None:
                        for bc in range(NBC):
                            # each stripe gets its own bank: start=True
                            # clears has_written bank-wide, so stripes must
                            # not share one
                            nc.tensor.matmul(
                                pls[bc][32 * bc:32 * bc + C, :],
                                w2sb[:, (hc - 1) * C:hc * C], prev_hT[bc][:],
                                start=(hc == 1), stop=False,
                                tile_position=(0, 32 * bc))
                    cur = []
                    for bc in range(NBC):
                        hT = hp.tile([128, 512], BF16, tag="hT", name="hT",
                                     bufs=8)
                        nc.scalar.activation(hT[:], phs[bc][:], AF.Gelu,
                                             bias=b1sb[:, hc:hc + 1])
                        cur.append(hT)
                    prev_hT = cur
                for bc in range(NBC):
                    nc.tensor.matmul(
                        pls[bc][32 * bc:32 * bc + C, :],
                        w2sb[:, (NHC - 1) * C:NHC * C],
                        prev_hT[bc][:], start=False, stop=True,
                        tile_position=(0, 32 * bc))

                for bc in range(NBC):
                    csl = slice(bc * 512, (bc + 1) * 512)
                    psl = slice(32 * bc, 32 * bc + C)
                    if e == 0:
                        nc.vector.scalar_tensor_tensor(
                            accT[psl, csl], pls[bc][psl, :],
                            b2sb[psl, e:e + 1],
                            gexps[e][psl, csl], ALU.add, ALU.mult)
                    else:
                        lt = hp.tile([128, 512], F32, tag="lt", bufs=2)
                        nc.vector.scalar_tensor_tensor(
                            lt[psl, :], pls[bc][psl, :], b2sb[psl, e:e + 1],
                            gexps[e][psl, csl], ALU.add, ALU.mult)
                        nc.vector.tensor_add(accT[psl, csl], accT[psl, csl],
                                             lt[psl, :])

                if e == 0:
                    # replicate 1/sum to C partitions; rin is long since
                    # ready, so these tiny matmuls cost no PE stall here
                    for bc in range(NBC):
                        csl = slice(bc * 512, (bc + 1) * 512)
                        prr = php.tile([99, 512], F32, tag="mm", name="prr")
                        nc.tensor.matmul(
                            prr[:], ones13[:, bc * 128:bc * 128 + 99],
                            rin[:, csl], start=True, stop=True)
                        nc.vector.tensor_copy(rrep[0:99, csl], prr[:])

            # softmax denominator applied per chunk right after the last add
            for bc in range(NBC):
                csl = slice(bc * 512, (bc + 1) * 512)
                psl = slice(32 * bc, 32 * bc + C)
                nc.vector.tensor_mul(accT[psl, csl], accT[psl, csl],
                                     rrep[psl, csl])
                nc.sync.dma_start(y[:, csl], accT[psl, csl])

    if LDW_DEDUP:
        _dedup_ldweights(nc)
    nc.finalize()
    return nc


def _fold_inputs(inputs):
    x = np.asarray(inputs["x"], np.float32)
    gg = np.asarray(inputs["gate_ln_g"], np.float32)
    gbeta = np.asarray(inputs["gate_ln_b"], np.float32)
    gw_ = np.asarray(inputs["gate_w"], np.float32)
    gbias = np.asarray(inputs["gate_b"], np.float32)
    eg = np.asarray(inputs["ex_ln_g"], np.float32)
    eb = np.asarray(inputs["ex_ln_b"], np.float32)
    w1_ = np.asarray(inputs["ex_w1"], np.float32)
    b1_ = np.asarray(inputs["ex_b1"], np.float32)
    w2_ = np.asarray(inputs["ex_w2"], np.float32)
    b2_ = np.asarray(inputs["ex_b2"], np.float32)

    # fold the (shared-normalize, per-head affine) LayerNorms into the
    # following linear layers: (n*g+b) @ W == n @ (g[:,None]*W) + b@W
    gwf = (gg[:, None] * gw_).astype(np.float32)                    # [D, E]
    gbf = (gbias + gbeta @ gw_).astype(np.float32)                  # [E]
    w1f = (eg[:, :, None] * w1_).astype(np.float32)                 # [E, D, H]
    b1f = (b1_ + np.einsum("ed,edh->eh", eb, w1_)).astype(np.float32)

    gw_dev = np.ascontiguousarray(
        gwf.reshape(KC, 128, E).transpose(1, 0, 2)).astype(BFNP)
    gb_dev = np.ascontiguousarray(gbf.reshape(E, 1))
    w1_dev = np.ascontiguousarray(
        w1f.reshape(E, KC, 128, H).transpose(0, 2, 1, 3)).astype(BFNP)
    b1_dev = np.ascontiguousarray(b1f.reshape(E, NHC, 128).transpose(0, 2, 1))
    w2_dev = np.ascontiguousarray(
        w2_.reshape(E, NHC, 128, C).transpose(0, 2, 1, 3).reshape(
            E, 128, NHC * C)).astype(BFNP)
    b2_dev = np.zeros((128, E), np.float32)
    rep_dev = np.zeros((E, E * NBC * 128), np.float32)
    on13_dev = np.zeros((1, NBC * 128), np.float32)
    for bc in range(NBC):
        b2_dev[32 * bc:32 * bc + C, :] = b2_.T
        for e in range(E):
            off = (e * NBC + bc) * 128 + 32 * bc
            rep_dev[e, off:off + C] = 1.0
        on13_dev[0, bc * 128 + 32 * bc:bc * 128 + 32 * bc + C] = 1.0
    weights = dict(gw=gw_dev, gb=gb_dev, w1=w1_dev, b1=b1_dev,
                   w2=w2_dev, b2=b2_dev, rep=rep_dev,
                   on4=np.ones((E, 1), np.float32), on13=on13_dev)
    return x.astype(BFNP), weights


def _get_nc():
    if "nc" not in _NC_CACHE:
        _NC_CACHE["nc"] = _build_nc()
    return _NC_CACHE["nc"]


def _in_maps(inputs):
    x, weights = _fold_inputs(inputs)
    maps = []
    for c in range(N_CORES):
        m = dict(weights)
        m["x"] = np.ascontiguousarray(x[c * BS:(c + 1) * BS])
        maps.append(m)
    return maps


def kernel(**inputs) -> np.ndarray:
    nc = _get_nc()
    maps = _in_maps(inputs)
    res = run_bass_kernel_spmd(nc, maps, list(range(N_CORES))).results
    out = np.empty((B, C), np.float32)
    for c in range(N_CORES):
        out[c * BS:(c + 1) * BS] = res[c]["y"].T
    return out


# revision 54
# speedup vs baseline: 1.0202x; 1.0202x over previous
"""Trainium2 Bass kernel for an MoE classification head.

Model (per reference):
    normed = LayerNorm(x)  (no affine; shared across gate+experts)
    gate   = softmax((normed * g_g + b_g) @ gate_w + gate_b)     [B, E]
    h_e    = GELU((normed * g_e + b_e) @ w1_e + b1_e)            [E, B, H]
    out    = sum_e gate[:, e] * (h_e @ w2_e + b2_e)              [B, C]

Strategy: data-parallel over 8 NeuronCores (batch sharded 2048 rows/core,
all parameters replicated).  Per-expert LayerNorm affines fold into w1/b1
on the host; everything runs in bf16 on the PE (error budget 2e-2 >> bf16
noise ~5e-3).  Key device-side structure:

  phase 0  per 128-row tile: DMA x (bf16) -> bn_stats/bn_aggr for LN
           stats -> scalar normalize -> XBAR DMA-transpose into
           normedT [128, KC, BS] (no PE transposes, no PSUM copies).
  gate     transposed logits [E, 512] via PE (stationary gw, moving
           normedT), exp on scalar, Sum/replication via tiny PE matmuls
           (ones / replication matrices); softmax denominator applied at
           the very end (out *= 1/sum replicated to C partitions).
  experts  for e, hc: mm1 accumulates KC chunks into 4 psum banks
           (batch chunks of 512); GELU+b1 on scalar -> bf16 hT; mm2
           accumulates over hc into [C, 512] psum.  Gated sum via one
           fused scalar_tensor_tensor: (mm2 + b2) * gate_exp.

A post-scheduling pass dedups consecutive identical LDWEIGHTS (the 4
batch-chunk matmuls per weight tile share one weight load).
"""

import os
import numpy as np
import ml_dtypes

import concourse.bacc as bacc
import concourse.mybir as mybir
from concourse import tile
from concourse.bass_utils import run_bass_kernel_spmd

F32 = mybir.dt.float32
F32R = mybir.dt.float32r
BF16 = mybir.dt.bfloat16
AF = mybir.ActivationFunctionType
ALU = mybir.AluOpType

N_CORES = 8
B, D, H, E, C = 16384, 1024, 2048, 4, 3
BS = B // N_CORES       # 2048 rows per core
NT = BS // 128          # 16 batch tiles of 128 rows
KC = D // 128           # 8 contraction chunks over D
NBC = BS // 512         # 4 batch chunks of 512 (matmul moving dim)
NHC = H // 128          # 16 H chunks
EPS = 1e-5
BFNP = ml_dtypes.bfloat16

_NC_CACHE = {}
# HW-unsafe: with one LDWEIGHTS per 4 matmuls the PE reorder window pulls
# the next weight load ahead and clobbers the array for the later matmuls
# of the group (bc2/bc3 outputs corrupt).  Measured benefit was ~0.3%, the
# reorder window already hides duplicate loads.  Keep off.
LDW_DEDUP = os.environ.get("BASS_LDW_DEDUP", "") != ""


def _dedup_ldweights(nc):
    """Remove back-to-back InstLdweights that reload identical weights.

    The tile scheduler splits every matmul into LDWEIGHTS + MATMUL; when
    consecutive matmuls share the same stationary operand (our 4 batch
    chunks per weight tile) the repeat loads are pure overhead on the PE.
    Only sync-free duplicates are dropped, so all semaphore waits are
    preserved.
    """
    removed = 0
    for f in nc.m.functions:
        for blk in f.blocks:
            insts = list(blk.instructions)
            keep = []
            last_sig = None
            pending_waits = []
            changed = False
            for i in insts:
                if isinstance(i, mybir.InstLdweights):
                    si = i.sync_info
                    clean = si is None or (
                        len(si.on_wait) == 0 and len(si.on_update) == 0
                    )
                    sig = (
                        str(i.ins),
                        str(i.perf_mode),
                        str(i.is_transpose),
                        str(i.tile_position),
                    )
                    if clean and sig == last_sig:
                        removed += 1
                        changed = True
                        continue
                    last_sig = sig
                keep.append(i)
            if changed:
                blk.instructions = keep
    return removed


def _build_nc():
    nc = bacc.Bacc("TRN2", target_bir_lowering=False, debug=False,
                   enable_asserts=False, num_devices=N_CORES)
    x = nc.dram_tensor("x", [BS, D], BF16, kind="ExternalInput")
    gw = nc.dram_tensor("gw", [128, KC, E], BF16, kind="ExternalInput")
    gb = nc.dram_tensor("gb", [E, 1], F32, kind="ExternalInput")
    w1 = nc.dram_tensor("w1", [E, 128, KC, H], BF16, kind="ExternalInput")
    b1 = nc.dram_tensor("b1", [E, 128, NHC], F32, kind="ExternalInput")
    w2 = nc.dram_tensor("w2", [E, 128, NHC * C], BF16, kind="ExternalInput")
    b2 = nc.dram_tensor("b2", [128, E], F32, kind="ExternalInput")
    rep = nc.dram_tensor("rep", [E, E * NBC * 128], F32, kind="ExternalInput")
    on4 = nc.dram_tensor("on4", [E, 1], F32, kind="ExternalInput")
    on13 = nc.dram_tensor("on13", [1, NBC * 128], F32, kind="ExternalInput")
    y = nc.dram_tensor("y", [C, BS], F32, kind="ExternalOutput")

    with tile.TileContext(nc) as tc:
        with (
            tc.tile_pool(name="pers", bufs=1) as pers,
            tc.tile_pool(name="xp", bufs=3) as xp,
            tc.tile_pool(name="nrmp", bufs=2) as nrmp,
            tc.tile_pool(name="st", bufs=3) as st,
            tc.tile_pool(name="gp", bufs=2) as gp,
            tc.tile_pool(name="w1p", bufs=2) as w1p,
            tc.tile_pool(name="ep", bufs=2) as ep,
            tc.tile_pool(name="hp", bufs=4) as hp,
            tc.tile_pool(name="php", bufs=5, space="PSUM") as php,
            tc.tile_pool(name="plp", bufs=3, space="PSUM") as plp,
        ):
            # ---- persistent tiles ----
            normedT = pers.tile([128, KC, BS], BF16)  # normalized x, transposed
            gwsb = pers.tile([128, KC, E], BF16)
            gbsb = pers.tile([E, 1], F32)
            b2sb = pers.tile([128, E], F32)           # b2 striped at 32*bc+c
            accT = pers.tile([128, BS], F32)          # gated sum, striped
            gexps = [pers.tile([128, BS], BF16, name=f"gexp{e}")
                     for e in range(E)]               # exp(logit_e), striped
            rin = pers.tile([1, BS], F32R)            # 1/sum_e exp
            rrep = pers.tile([128, BS], F32)          # rin striped to 32*bc+c
            epst = pers.tile([128, 1], F32)
            ones4 = pers.tile([E, 1], F32R)           # column of ones (sum_e)
            ones13 = pers.tile([1, NBC * 128], F32R)  # striped 1s per bc
            repm = pers.tile([E, E * NBC * 128], F32R)  # striped replication

            nc.vector.memset(epst[:], EPS)
            nc.gpsimd.memset(accT[:], 0.0)
            nc.gpsimd.memset(rrep[:], 0.0)
            nc.scalar.dma_start(ones4[:], on4[:].bitcast(F32R))
            nc.scalar.dma_start(ones13[:], on13[:].bitcast(F32R))
            nc.scalar.dma_start(repm[:], rep[:].bitcast(F32R))
            nc.scalar.dma_start(gwsb[:], gw[:])
            nc.scalar.dma_start(gbsb[:], gb[:])
            nc.scalar.dma_start(b2sb[:], b2[:])

            # prefetch expert 0 weights (gpsimd queue is otherwise idle)
            w1sb0 = w1p.tile([128, KC, H], BF16, tag="w1", name="w1sb0")
            nc.gpsimd.dma_start(w1sb0[:], w1[0])
            b1sb0 = ep.tile([128, NHC], F32, tag="b1", name="b1sb0")
            nc.gpsimd.dma_start(b1sb0[:], b1[0])
            w2sb0 = ep.tile([128, NHC * C], BF16, tag="w2", name="w2sb0")
            nc.gpsimd.dma_start(w2sb0[:], w2[0])

            # ---- phase 0: LayerNorm stats + normalize + DMA-transpose ----
            # two row-tiles per load so the tiny stats ops batch 2 wide
            for tp in range(NT // 2):
                r0 = 256 * tp
                xt = xp.tile([128, 2, D], BF16, tag="xt", name="xt", bufs=5)
                nc.sync.dma_start(
                    xt[:], x[r0:r0 + 256, :].rearrange("(a p) d -> p a d",
                                                       p=128))
                mvp = st.tile([128, 2, 2], F32, tag="mvp")
                for i in range(2):
                    bnst = st.tile([128, 12], F32, tag="bnst")
                    nc.vector.bn_stats(bnst[:, 0:6], xt[:, i, 0:512])
                    nc.vector.bn_stats(bnst[:, 6:12], xt[:, i, 512:1024])
                    nc.vector.bn_aggr(mvp[:, i, :], bnst[:])
                sdp = st.tile([128, 2], F32, tag="sdp")
                nc.scalar.activation(sdp[:], mvp[:, :, 1], AF.Sqrt,
                                     bias=epst[:])
                rsp = st.tile([128, 2], F32, tag="rsp")
                nc.vector.reciprocal(rsp[:], sdp[:])
                nmp = st.tile([128, 2], F32, tag="nmp")
                nc.vector.scalar_tensor_tensor(
                    nmp[:], mvp[:, :, 0], -1.0, rsp[:], ALU.mult, ALU.mult)
                for i in range(2):
                    ti = 2 * tp + i
                    bsl = slice(ti * 128, (ti + 1) * 128)
                    nrm = nrmp.tile([128, D], BF16, tag="nrm", name="nrm",
                                    bufs=8)
                    nc.scalar.activation(nrm[:], xt[:, i, :], AF.Identity,
                                         bias=nmp[:, i:i + 1],
                                         scale=rsp[:, i:i + 1])
                    # [128b, 1024d] -> normedT[dp, kc, b] via XBAR transpose
                    nc.sync.dma_start(normedT[:, :, bsl], nrm[:],
                                      transpose=True)

            # ---- gate: logitsT -> exp -> sum + replicate (PE) ----
            # batch the PE work so the in-order PE queue never stalls on the
            # scalar/vector softmax chain.  gb2 = gbsb + 0*normedT[last tile]
            # pins the exps after ALL phase-0 scalar work, so the act table
            # switches sqrt-set -> exp-set -> gelu-set exactly once each.
            gb2 = pers.tile([E, 1], F32)
            nc.vector.scalar_tensor_tensor(
                gb2[:], normedT[0:E, KC - 1, BS - 1:BS], 0.0, gbsb[:],
                ALU.mult, ALU.add)
            pgs, exgs = [], []
            for bc in range(NBC):
                csl = slice(bc * 512, (bc + 1) * 512)
                pg = php.tile([E, 512], F32, tag="mm", name="pg")
                for kc in range(KC):
                    nc.tensor.matmul(pg[:], gwsb[:, kc, :], normedT[:, kc, csl],
                                     start=(kc == 0), stop=(kc == KC - 1))
                pgs.append(pg)
            for bc in range(NBC):
                exg = gp.tile([E, 512], F32R, tag="exg", name="exg", bufs=4)
                nc.scalar.activation(exg[:], pgs[bc][:], AF.Exp, bias=gb2[:])
                exgs.append(exg)
            for bc in range(NBC):
                csl = slice(bc * 512, (bc + 1) * 512)
                ps1 = plp.tile([1, 512], F32, tag="pl", name="ps1")
                nc.tensor.matmul(ps1[:], ones4[:], exgs[bc][:],
                                 start=True, stop=True)
                with nc.allow_low_precision(reason="f32r output is f32 bits"):
                    nc.vector.reciprocal(rin[:, csl], ps1[:])
                for e in range(E):
                    off = (e * NBC + bc) * 128
                    pool = php if e % 2 == 0 else plp
                    tag = "mm" if e % 2 == 0 else "pl"
                    pr = pool.tile([99, 512], F32, tag=tag, name="pr")
                    nc.tensor.matmul(
                        pr[:], repm[:, off:off + 99],
                        exgs[bc][:], start=True, stop=True)
                    nc.vector.tensor_copy(gexps[e][0:99, csl], pr[:])

            # ---- experts ----
            for e in range(E):
                if e == 0:
                    w1sb, b1sb, w2sb = w1sb0, b1sb0, w2sb0
                else:
                    w1sb = w1p.tile([128, KC, H], BF16, tag="w1")
                    nc.gpsimd.dma_start(w1sb[:], w1[e])
                    b1sb = ep.tile([128, NHC], F32, tag="b1")
                    nc.gpsimd.dma_start(b1sb[:], b1[e])
                    w2sb = ep.tile([128, NHC * C], BF16, tag="w2")
                    nc.gpsimd.dma_start(w2sb[:], w2[e])

                # two stripes per mm2 bank: the banks are zeroed by DVE
                # memset and every mm2 runs start=False, so add-onto-zero
                # and overwrite are both correct whatever the stale
                # has_written bits say.  Frees two PSUM banks for php.
                plsAB = [plp.tile([99, 512], F32, tag="pl", name="pl")
                         for _ in range(2)]
                nc.vector.memset(plsAB[0][:], 0.0)
                nc.vector.memset(plsAB[1][:], 0.0)
                # mm2 for iteration hc is issued during mm1 of hc+1, so it
                # never waits on a just-issued GELU at the PE queue head
                prev_hT = None
                for hc in range(NHC):
                    hsl = slice(hc * 128, (hc + 1) * 128)
                    phs = [php.tile([128, 512], F32, tag="mm", name="ph")
                           for _ in range(NBC)]
                    for kc in range(KC):
                        for bc in range(NBC):
                            nc.tensor.matmul(
                                phs[bc][:], w1sb[:, kc, hsl],
                                normedT[:, kc, bc * 512:(bc + 1) * 512],
                                start=(kc == 0), stop=(kc == KC - 1))
                    if prev_hT is not None:
                        for bc in range(NBC):
                            nc.tensor.matmul(
                                plsAB[bc // 2][32 * bc:32 * bc + C, :],
                                w2sb[:, (hc - 1) * C:hc * C], prev_hT[bc][:],
                                start=False, stop=False,
                                skip_group_check=True,
                                tile_position=(0, 32 * bc))
                    cur = []
                    for bc in range(NBC):
                        hT = hp.tile([128, 512], BF16, tag="hT", name="hT",
                                     bufs=8)
                        nc.scalar.activation(hT[:], phs[bc][:], AF.Gelu,
                                             bias=b1sb[:, hc:hc + 1])
                        cur.append(hT)
                    prev_hT = cur
                for bc in range(NBC):
                    nc.tensor.matmul(
                        plsAB[bc // 2][32 * bc:32 * bc + C, :],
                        w2sb[:, (NHC - 1) * C:NHC * C],
                        prev_hT[bc][:], start=False, stop=True,
                        skip_group_check=True,
                        tile_position=(0, 32 * bc))

                for bc in range(NBC):
                    csl = slice(bc * 512, (bc + 1) * 512)
                    psl = slice(32 * bc, 32 * bc + C)
                    if e == 0:
                        nc.vector.scalar_tensor_tensor(
                            accT[psl, csl], plsAB[bc // 2][psl, :],
                            b2sb[psl, e:e + 1],
                            gexps[e][psl, csl], ALU.add, ALU.mult)
                    else:
                        lt = hp.tile([128, 512], F32, tag="lt", bufs=2)
                        nc.vector.scalar_tensor_tensor(
                            lt[psl, :], plsAB[bc // 2][psl, :],
                            b2sb[psl, e:e + 1],
                            gexps[e][psl, csl], ALU.add, ALU.mult)
                        nc.vector.tensor_add(accT[psl, csl], accT[psl, csl],
                                             lt[psl, :])

                if e == 0:
                    # replicate 1/sum to C partitions; rin is long since
                    # ready, so these tiny matmuls cost no PE stall here
                    for bc in range(NBC):
                        csl = slice(bc * 512, (bc + 1) * 512)
                        prr = php.tile([99, 512], F32, tag="mm", name="prr")
                        nc.tensor.matmul(
                            prr[:], ones13[:, bc * 128:bc * 128 + 99],
                            rin[:, csl], start=True, stop=True)
                        nc.vector.tensor_copy(rrep[0:99, csl], prr[:])

            # softmax denominator applied per chunk right after the last add
            for bc in range(NBC):
                csl = slice(bc * 512, (bc + 1) * 512)
                psl = slice(32 * bc, 32 * bc + C)
                nc.vector.tensor_mul(accT[psl, csl], accT[psl, csl],
                                     rrep[psl, csl])
                nc.sync.dma_start(y[:, csl], accT[psl, csl])

    if LDW_DEDUP:
        _dedup_ldweights(nc)
    nc.finalize()
    return nc


def _fold_inputs(inputs):
    x = np.asarray(inputs["x"], np.float32)
    gg = np.asarray(inputs["gate_ln_g"], np.float32)
    gbeta = np.asarray(inputs["gate_ln_b"], np.float32)
    gw_ = np.asarray(inputs["gate_w"], np.float32)
    gbias = np.asarray(inputs["gate_b"], np.float32)
    eg = np.asarray(inputs["ex_ln_g"], np.float32)
    eb = np.asarray(inputs["ex_ln_b"], np.float32)
    w1_ = np.asarray(inputs["ex_w1"], np.float32)
    b1_ = np.asarray(inputs["ex_b1"], np.float32)
    w2_ = np.asarray(inputs["ex_w2"], np.float32)
    b2_ = np.asarray(inputs["ex_b2"], np.float32)

    # fold the (shared-normalize, per-head affine) LayerNorms into the
    # following linear layers: (n*g+b) @ W == n @ (g[:,None]*W) + b@W
    gwf = (gg[:, None] * gw_).astype(np.float32)                    # [D, E]
    gbf = (gbias + gbeta @ gw_).astype(np.float32)                  # [E]
    w1f = (eg[:, :, None] * w1_).astype(np.float32)                 # [E, D, H]
    b1f = (b1_ + np.einsum("ed,edh->eh", eb, w1_)).astype(np.float32)

    gw_dev = np.ascontiguousarray(
        gwf.reshape(KC, 128, E).transpose(1, 0, 2)).astype(BFNP)
    gb_dev = np.ascontiguousarray(gbf.reshape(E, 1))
    w1_dev = np.ascontiguousarray(
        w1f.reshape(E, KC, 128, H).transpose(0, 2, 1, 3)).astype(BFNP)
    b1_dev = np.ascontiguousarray(b1f.reshape(E, NHC, 128).transpose(0, 2, 1))
    w2_dev = np.ascontiguousarray(
        w2_.reshape(E, NHC, 128, C).transpose(0, 2, 1, 3).reshape(
            E, 128, NHC * C)).astype(BFNP)
    b2_dev = np.zeros((128, E), np.float32)
    rep_dev = np.zeros((E, E * NBC * 128), np.float32)
    on13_dev = np.zeros((1, NBC * 128), np.float32)
    for bc in range(NBC):
        b2_dev[32 * bc:32 * bc + C, :] = b2_.T
        for e in range(E):
            off = (e * NBC + bc) * 128 + 32 * bc
            rep_dev[e, off:off + C] = 1.0
        on13_dev[0, bc * 128 + 32 * bc:bc * 128 + 32 * bc + C] = 1.0
    weights = dict(gw=gw_dev, gb=gb_dev, w1=w1_dev, b1=b1_dev,
                   w2=w2_dev, b2=b2_dev, rep=rep_dev,
                   on4=np.ones((E, 1), np.float32), on13=on13_dev)
    return x.astype(BFNP), weights


def _get_nc():
    if "nc" not in _NC_CACHE:
        _NC_CACHE["nc"] = _build_nc()
    return _NC_CACHE["nc"]


def _in_maps(inputs):
    x, weights = _fold_inputs(inputs)
    maps = []
    for c in range(N_CORES):
        m = dict(weights)
        m["x"] = np.ascontiguousarray(x[c * BS:(c + 1) * BS])
        maps.append(m)
    return maps


def kernel(**inputs) -> np.ndarray:
    nc = _get_nc()
    maps = _in_maps(inputs)
    res = run_bass_kernel_spmd(nc, maps, list(range(N_CORES))).results
    out = np.empty((B, C), np.float32)
    for c in range(N_CORES):
        out[c * BS:(c + 1) * BS] = res[c]["y"].T
    return out
